# revision 16
# baseline (speedup 1.0000x reference)
"""Trainium2 Bass kernel for nn_AugmentWithTrace (gnn_message_passing).

Reference computation:
    g = trace_pool[neighbor_idx]                       # [T, K, D] gather
    s = MLP3(g)                                        # per-row scores
    attn = masked_softmax_k(s)                         # over K=8 neighbors
    out = einsum('tk,tkd->td', attn, g)                # [T, D]

Key optimizations over the naive structure:
  - neighbor_mask is ~50% dense: tokens are sorted by valid-neighbor count
    on the host and packed into 128-token chunks; chunk l only processes
    K_l = max count in that chunk slots (avg ~4 instead of 8). The sorted
    chunk schedule is striped across the 8 cores; the compiled program
    bakes the per-chunk K schedule (identical for all cores = max over the
    stripe).
  - trace_pool is cast to bf16 on the host and compacted per-core to the
    ~16k rows that core actually references, so indices fit int16 and the
    SWDGE dma_gather extended instruction can be used.
  - Per group of 4 chunks, TWO dma_gather calls fetch the rows: one
    transposed (d on partitions -> direct matmul rhs, no PE transposes, no
    bf16 cast, no PSUM->SBUF copies) and one row-major (for the weighted
    sum).
  - 3-layer MLP in bf16 (f32 PSUM accumulate). L3 lands scores directly as
    [token, k] via per-k matmuls with h2 slices as lhsT and w3 as a 1-col
    rhs.
  - Softmax over k with exact-zero masking (no max subtraction; exp on ACT
    from SBUF - exp-from-PSUM kills the device).
  - Weighted sum on the PE: per slot k, lhsT = diag(attn[:,k]) built by one
    DVE/Pool tensor_scalar from the identity, rhs = row-major gathered
    slab; accumulate over k in PSUM.
"""

import sys

if "/opt/trn_rl_repo" not in sys.path:
    sys.path.insert(0, "/opt/trn_rl_repo")

import numpy as np
import ml_dtypes

T, K, D, N_POOL = 32768, 8, 256, 131072
N_CORES = 8
T_LOC = T // N_CORES          # 4096 tokens per core
N_CHUNK = T_LOC // 128        # 32 chunks of 128 tokens
N_CPOOL = 20480               # compacted per-core pool rows (padded)
GROUP = 2                     # chunks per dma_gather call

_CACHE = {}


def _build_kernel(b3_val, kls, stage=9,
                  diag_eng="dve", relu2_j1_eng="dve", acccopy_eng="dve",
                  group=GROUP, single_packet=False, relu1_j1_eng="act",
                  sccopy_eng="dve", diag_split=False, pipeline=True,
                  depth=1, pmm_bufs=4, psc_bufs=2, grp_bufs=2, gtp_bufs=2):
    import concourse.bass as bass
    import concourse.bacc as bacc
    import concourse.mybir as mybir
    import concourse.tile as tile
    from concourse.masks import make_identity

    f32 = mybir.dt.float32
    bf16 = mybir.dt.bfloat16
    i16 = mybir.dt.int16

    kls = list(kls)
    assert len(kls) == N_CHUNK
    tot_idx = sum(k * 128 for k in kls)

    nc = bacc.Bacc("TRN2", target_bir_lowering=False, debug=False,
                   num_devices=N_CORES)

    pool_d = nc.declare_dram_parameter("pool", [N_CPOOL, D], bf16, isOutput=False)
    idx_d = nc.declare_dram_parameter("idx", [128, max(tot_idx // 16, 1)], i16,
                                      isOutput=False)
    maskc_d = nc.declare_dram_parameter("maskc", [128, N_CHUNK * K], f32,
                                        isOutput=False)
    w1_d = nc.declare_dram_parameter("w1", [128, 512], bf16, isOutput=False)
    w2_d = nc.declare_dram_parameter("w2", [128, 512], bf16, isOutput=False)
    w3_d = nc.declare_dram_parameter("w3c", [128, 2], bf16, isOutput=False)
    b1_d = nc.declare_dram_parameter("b1c", [128, 2], f32, isOutput=False)
    b2_d = nc.declare_dram_parameter("b2c", [128, 2], f32, isOutput=False)
    out_d = nc.declare_dram_parameter("out", [T_LOC, D], bf16, isOutput=True)

    relu = mybir.ActivationFunctionType.Relu
    expf = mybir.ActivationFunctionType.Exp
    mult = mybir.AluOpType.mult
    amax = mybir.AluOpType.max
    aadd = mybir.AluOpType.add

    # group boundaries: [(chunk_lo, chunk_hi, idx_offset, n_idx), ...]
    groups = []
    off = 0
    for g0 in range(0, N_CHUNK, group):
        chunks = list(range(g0, min(g0 + group, N_CHUNK)))
        n_g = sum(kls[l] * 128 for l in chunks)
        groups.append((chunks, off, n_g))
        off += n_g
    assert off == tot_idx

    with tile.TileContext(nc) as tc:
        with (
            tc.tile_pool(name="const", bufs=1) as cp,
            tc.tile_pool(name="gT", bufs=gtp_bufs) as gtp,
            tc.tile_pool(name="gR", bufs=grp_bufs) as grp,
            tc.tile_pool(name="hid", bufs=3) as hp,
            tc.tile_pool(name="sml", bufs=3) as sp,
            tc.tile_pool(name="dg", bufs=3) as dgp,
            tc.tile_pool(name="outp", bufs=3) as op_,
            tc.tile_pool(name="pmm", bufs=pmm_bufs, space="PSUM") as pmm,
            tc.tile_pool(name="pacc", bufs=2, space="PSUM") as pacc,
            tc.tile_pool(name="psc", bufs=psc_bufs, space="PSUM") as pscp,
        ):
            # ---- constants loaded once ----
            idx_t = cp.tile([128, max(tot_idx // 16, 1)], i16)
            nc.sync.dma_start(out=idx_t[:], in_=idx_d[:])
            maskc_t = cp.tile([128, N_CHUNK * K], f32)
            nc.sync.dma_start(out=maskc_t[:], in_=maskc_d[:])
            w1_t = cp.tile([128, 512], bf16)
            nc.sync.dma_start(out=w1_t[:], in_=w1_d[:])
            w2_t = cp.tile([128, 512], bf16)
            nc.sync.dma_start(out=w2_t[:], in_=w2_d[:])
            w3_t = cp.tile([128, 2], bf16)
            nc.sync.dma_start(out=w3_t[:], in_=w3_d[:])
            b1_t = cp.tile([128, 2], f32)
            nc.sync.dma_start(out=b1_t[:], in_=b1_d[:])
            b2_t = cp.tile([128, 2], f32)
            nc.sync.dma_start(out=b2_t[:], in_=b2_d[:])
            ident = cp.tile([128, 128], bf16)
            make_identity(nc, ident[:])
            zero_t = cp.tile([128, D], bf16)
            nc.vector.memset(zero_t[:], 0.0)

            pending = []
            for (chunks, goff, n_g) in groups:
                if n_g == 0:
                    for l in chunks:
                        nc.sync.dma_start(out=out_d[l * 128:(l + 1) * 128, :],
                                          in_=zero_t[:])
                    continue
                # ---- group gathers ----
                gT = gtp.tile([128, 2, n_g], bf16, name=f"gT{goff}", tag="gT")
                nc.gpsimd.dma_gather(
                    gT[:], pool_d[:], idx_t[:, goff // 16:(goff + n_g) // 16],
                    n_g, n_g, D, transpose=True, single_packet=single_packet)
                gR = grp.tile([128, n_g // 128, D], bf16, name=f"gR{goff}",
                              tag="gR")
                nc.gpsimd.dma_gather(
                    gR[:], pool_d[:], idx_t[:, goff // 16:(goff + n_g) // 16],
                    n_g, n_g, D, single_packet=single_packet)

                off_l = 0
                for l in chunks:
                    kl = kls[l]
                    if kl == 0:
                        nc.sync.dma_start(out=out_d[l * 128:(l + 1) * 128, :],
                                          in_=zero_t[:])
                        continue
                    nl = kl * 128
                    if stage == 1:
                        ob = op_.tile([128, D], bf16, name=f"ob{l}", tag="ob")
                        nc.vector.tensor_copy(out=ob[:], in_=gR[:, off_l // 128, :])
                        nc.sync.dma_start(out=out_d[l * 128:(l + 1) * 128, :],
                                          in_=ob[:])
                        off_l += nl
                        continue

                    # ---- layer 1: h1[j,(k,c)] = relu(sum_h w1.T xt) ----
                    h1 = hp.tile([128, 2, nl], bf16, name=f"h1_{l}", tag="h1")
                    for j in range(2):
                        ps1 = pmm.tile([128, 512], f32, name=f"ps1_{l}{j}0", tag="mm")
                        pieces = [(ps1, 0, min(nl, 512))]
                        if nl > 512:
                            ps1b = pmm.tile([128, 512], f32, name=f"ps1_{l}{j}1",
                                            tag="mm")
                            pieces.append((ps1b, 512, nl - 512))
                        for (pst, n0, nn) in pieces:
                            for h in range(2):
                                nc.tensor.matmul(
                                    out=pst[:, :nn],
                                    lhsT=w1_t[:, h * 256 + j * 128:h * 256 + (j + 1) * 128],
                                    rhs=gT[:, h, off_l + n0:off_l + n0 + nn],
                                    start=(h == 0), stop=(h == 1))
                        for (pst, n0, nn) in pieces:
                            if j == 1 and relu1_j1_eng == "dve":
                                nc.vector.tensor_scalar(
                                    out=h1[:, j, n0:n0 + nn], in0=pst[:, :nn],
                                    scalar1=b1_t[:, j:j + 1], scalar2=0.0,
                                    op0=aadd, op1=amax)
                            else:
                                nc.scalar.activation(
                                    out=h1[:, j, n0:n0 + nn], in_=pst[:, :nn],
                                    func=relu, bias=b1_t[:, j:j + 1], scale=1.0)
                    if stage == 2:
                        nn0 = min(nl, D)
                        ob = op_.tile([128, D], bf16, name=f"ob{l}", tag="ob")
                        nc.vector.memset(ob[:], 0.0)
                        nc.vector.tensor_copy(out=ob[:, :nn0], in_=h1[:, 0, 0:nn0])
                        nc.sync.dma_start(out=out_d[l * 128:(l + 1) * 128, :],
                                          in_=ob[:])
                        off_l += nl
                        continue

                    # ---- layer 2 ----
                    h2 = hp.tile([128, 2, nl], bf16, name=f"h2_{l}", tag="h2")
                    for j in range(2):
                        ps2 = pmm.tile([128, 512], f32, name=f"ps2_{l}{j}0", tag="mm")
                        pieces = [(ps2, 0, min(nl, 512))]
                        if nl > 512:
                            ps2b = pmm.tile([128, 512], f32, name=f"ps2_{l}{j}1",
                                            tag="mm")
                            pieces.append((ps2b, 512, nl - 512))
                        for (pst, n0, nn) in pieces:
                            for h in range(2):
                                nc.tensor.matmul(
                                    out=pst[:, :nn],
                                    lhsT=w2_t[:, h * 256 + j * 128:h * 256 + (j + 1) * 128],
                                    rhs=h1[:, h, n0:n0 + nn],
                                    start=(h == 0), stop=(h == 1))
                        for (pst, n0, nn) in pieces:
                            if j == 1 and relu2_j1_eng == "dve":
                                nc.vector.tensor_scalar(
                                    out=h2[:, j, n0:n0 + nn], in0=pst[:, :nn],
                                    scalar1=b2_t[:, j:j + 1], scalar2=0.0,
                                    op0=aadd, op1=amax)
                            else:
                                nc.scalar.activation(
                                    out=h2[:, j, n0:n0 + nn], in_=pst[:, :nn],
                                    func=relu, bias=b2_t[:, j:j + 1], scale=1.0)

                    # ---- layer 3: psc[c, k] = sum_e w3[e] h2[e,(k,c)] ----
                    psc = pscp.tile([128, K], f32, name=f"psc{l}", tag="psc")
                    for k in range(kl):
                        for j in range(2):
                            nc.tensor.matmul(
                                out=psc[:, k:k + 1],
                                lhsT=h2[:, j, k * 128:(k + 1) * 128],
                                rhs=w3_t[:, j:j + 1],
                                start=(j == 0), stop=(j == 1))

                    # ---- tail: softmax + weighted sum (emitted delayed) ----
                    def tail(l=l, kl=kl, psc=psc, h2=h2, gR=gR, off_l=off_l):
                        sc = sp.tile([128, K], f32, name=f"sc{l}", tag="sc")
                        if sccopy_eng == "act":
                            nc.scalar.copy(out=sc[:, :kl], in_=psc[:, :kl])
                        else:
                            nc.vector.tensor_copy(out=sc[:, :kl], in_=psc[:, :kl])
                        e_t = sp.tile([128, K], f32, name=f"e{l}", tag="e")
                        nc.scalar.activation(out=e_t[:, :kl], in_=sc[:, :kl],
                                             func=expf, bias=float(b3_val), scale=1.0)
                        em = sp.tile([128, K], f32, name=f"em{l}", tag="em")
                        nc.vector.tensor_tensor(
                            out=em[:, :kl], in0=e_t[:, :kl],
                            in1=maskc_t[:, l * K:l * K + kl], op=mult)
                        z_t = sp.tile([128, 1], f32, name=f"z{l}", tag="z")
                        nc.vector.reduce_sum(z_t[:], em[:, :kl],
                                             axis=mybir.AxisListType.X)
                        nc.vector.tensor_scalar_add(z_t[:], z_t[:], 1e-30)
                        r_t = sp.tile([128, 1], f32, name=f"r{l}", tag="r")
                        nc.vector.reciprocal(out=r_t[:], in_=z_t[:])

                        # weighted sum on PE: acc += diag(attn_k)^T @ g_k
                        acc = pacc.tile([128, 512], f32, name=f"acc{l}", tag="acc")
                        for k in range(kl):
                            if diag_split:
                                deng = nc.gpsimd if (k % 2 == 1) else nc.vector
                            else:
                                deng = nc.gpsimd if diag_eng == "pool" else nc.vector
                            dk = dgp.tile([128, 128], bf16, name=f"dk{l}_{k}", tag="dk")
                            deng.tensor_scalar(
                                out=dk[:], in0=ident[:],
                                scalar1=em[:, k:k + 1], scalar2=r_t[:, 0:1],
                                op0=mult, op1=mult)
                            nc.tensor.matmul(
                                out=acc[:, :D], lhsT=dk[:],
                                rhs=gR[:, off_l // 128 + k, :],
                                start=(k == 0), stop=(k == kl - 1))

                        ob = op_.tile([128, D], bf16, name=f"ob{l}", tag="ob")
                        if acccopy_eng == "dve":
                            nc.vector.tensor_copy(out=ob[:], in_=acc[:, :D])
                        else:
                            nc.scalar.copy(out=ob[:], in_=acc[:, :D])
                        nc.sync.dma_start(out=out_d[l * 128:(l + 1) * 128, :],
                                          in_=ob[:])

                    if pipeline:
                        pending.append(tail)
                        if len(pending) > depth:
                            pending.pop(0)()
                    else:
                        tail()
                    off_l += nl

                    if False and stage == 5:
                        # fallback weighted sum on DVE (baseline-style STT)
                        a_t = sp.tile([128, K], f32, name=f"a{l}", tag="a")
                        nc.vector.tensor_scalar_mul(a_t[:, :kl], em[:, :kl],
                                                    r_t[:, 0:1])
                        accs = op_.tile([128, D], f32, name=f"accs{l}", tag="accs")
                        nc.vector.tensor_scalar_mul(
                            accs[:], gR[:, off_l // 128, :], a_t[:, 0:1])
                        for k in range(1, kl):
                            nc.vector.scalar_tensor_tensor(
                                out=accs[:], in0=gR[:, off_l // 128 + k, :],
                                scalar=a_t[:, k:k + 1], in1=accs[:],
                                op0=mult, op1=aadd)
                        ob = op_.tile([128, D], bf16, name=f"ob{l}", tag="ob")
                        nc.vector.tensor_copy(out=ob[:], in_=accs[:])
                        nc.sync.dma_start(out=out_d[l * 128:(l + 1) * 128, :],
                                          in_=ob[:])
                        off_l += nl
                        continue


            for fn in pending:
                fn()

    nc.compile()
    return nc


def _host_prep(trace_pool, neighbor_idx, neighbor_mask, W1, b1, W2, b2, W3, b3):
    """Sort/compact on host; returns (kls, in_maps, order) for the SPMD run."""
    mask = np.asarray(neighbor_mask).astype(bool)
    nidx = np.asarray(neighbor_idx).astype(np.int64)
    counts = mask.sum(1)
    order = np.argsort(-counts, kind="stable")        # sorted token ids
    # global chunk g -> core g % 8, local chunk g // 8.
    # Per-local-chunk K = max count over the stripe = count of first token
    # of global chunk 8*l (descending sort).
    kls = [int(counts[order[(8 * l) * 128]]) for l in range(N_CHUNK)]
    tot_idx = sum(k * 128 for k in kls)

    pool_bf16 = np.ascontiguousarray(np.asarray(trace_pool, np.float32)
                                     ).astype(ml_dtypes.bfloat16)

    bfc = lambda x: np.ascontiguousarray(x).astype(ml_dtypes.bfloat16)
    W1 = np.asarray(W1, np.float32)
    W2 = np.asarray(W2, np.float32)
    W3 = np.asarray(W3, np.float32)
    b1 = np.asarray(b1, np.float32)
    b2 = np.asarray(b2, np.float32)
    w_shared = {
        "w1": bfc(W1.reshape(2, 128, 256).transpose(1, 0, 2).reshape(128, 512)),
        "w2": bfc(W2.reshape(2, 128, 256).transpose(1, 0, 2).reshape(128, 512)),
        "w3c": bfc(W3.reshape(2, 128).T),
        "b1c": np.ascontiguousarray(b1.reshape(2, 128).T).astype(np.float32),
        "b2c": np.ascontiguousarray(b2.reshape(2, 128).T).astype(np.float32),
    }

    in_maps = []
    for c in range(N_CORES):
        # tokens for local chunk l: order[(8l+c)*128 : (8l+c+1)*128]
        toks = np.stack([order[(8 * l + c) * 128:(8 * l + c + 1) * 128]
                         for l in range(N_CHUNK)])          # [N_CHUNK, 128]
        cnt = counts[toks]                                   # [N_CHUNK, 128]
        # compacted slot table: sidx[l, p, k] = pool idx of k-th valid
        # neighbor of token (l, p), padded with its slot-0 index (or 0).
        sidx = np.zeros((N_CHUNK, 128, K), np.int64)
        smask = np.zeros((N_CHUNK, 128, K), np.float32)
        for l in range(N_CHUNK):
            m = mask[toks[l]]                                # [128, K]
            ii = nidx[toks[l]]                               # [128, K]
            for p in range(128):
                v = ii[p, m[p]]
                nv = len(v)
                if nv:
                    sidx[l, p, :nv] = v
                    sidx[l, p, nv:] = v[0]
                    smask[l, p, :nv] = 1.0
        # unique rows this core touches (within the kls[l] slot ranges)
        used = np.concatenate(
            [sidx[l, :, :kls[l]].ravel() for l in range(N_CHUNK) if kls[l]]
        ) if tot_idx else np.zeros(1, np.int64)
        uniq, inv = np.unique(used, return_inverse=True)
        assert len(uniq) <= min(N_CPOOL, 32768), f"core {c}: {len(uniq)} uniq"
        pool_c = np.zeros((N_CPOOL, D), ml_dtypes.bfloat16)
        pool_c[:len(uniq)] = pool_bf16[uniq]
        # flat idx list in group gather order: chunks ascending, k-major
        # within chunk: j = chunk_off + k*128 + p
        flat = np.zeros(max(tot_idx, 16), np.int16)
        pos = 0
        ptr = 0
        for l in range(N_CHUNK):
            kl = kls[l]
            if kl == 0:
                continue
            n_l = 128 * kl
            cidx = inv[ptr:ptr + n_l].reshape(128, kl)       # [p, k]
            ptr += n_l
            flat[pos:pos + n_l] = cidx.T.reshape(-1).astype(np.int16)
            pos += n_l
        # wrap: element j -> partition j%16, column j//16; replicate x8
        ncol = max(tot_idx // 16, 1)
        wrapped = np.zeros((128, ncol), np.int16)
        w16 = flat[:ncol * 16].reshape(ncol, 16).T           # [16, ncol]
        wrapped[:, :] = np.tile(w16, (8, 1))
        # mask columns: maskc[p, l*8+k]
        maskc = np.zeros((128, N_CHUNK * K), np.float32)
        for l in range(N_CHUNK):
            maskc[:, l * K:(l + 1) * K] = smask[l]
        m = {"pool": pool_c, "idx": wrapped, "maskc": maskc}
        m.update(w_shared)
        in_maps.append(m)
    return kls, in_maps, order


def prepare(inputs):
    """Build (cached) kernel + per-core input maps. Returns
    (nc, in_maps, assemble_fn)."""
    b3_arr = np.asarray(inputs["b3"], dtype=np.float32)
    b3_val = float(b3_arr.reshape(-1)[0])
    kls, in_maps, order = _host_prep(**inputs)
    key = (b3_val, tuple(kls))
    if _CACHE.get("key") != key:
        _CACHE["nc"] = _build_kernel(b3_val, kls)
        _CACHE["key"] = key
    nc = _CACHE["nc"]

    def assemble(res):
        outs = np.concatenate([np.asarray(res[c]["out"]) for c in range(N_CORES)],
                              axis=0)  # [T] rows in (core, local) order
        # device row (c, l*128+p) = sorted position (8l+c)*128+p
        # -> token order[(8l+c)*128+p]
        out_full = np.zeros((T, D), np.float32)
        srt = outs.reshape(N_CORES, N_CHUNK, 128, D).transpose(1, 0, 2, 3) \
                  .reshape(T, D).astype(np.float32)
        out_full[order] = srt
        return out_full

    return nc, in_maps, assemble


def kernel(trace_pool, neighbor_idx, neighbor_mask, W1, b1, W2, b2, W3, b3):
    inputs = dict(trace_pool=trace_pool, neighbor_idx=neighbor_idx,
                  neighbor_mask=neighbor_mask, W1=W1, b1=b1, W2=W2, b2=b2,
                  W3=W3, b3=b3)
    nc, in_maps, assemble = prepare(inputs)

    from concourse.bass_utils import run_bass_kernel_spmd

    res = run_bass_kernel_spmd(nc, in_maps, core_ids=list(range(N_CORES)))
    return assemble(res.results)
